# revision 33
# baseline (speedup 1.0000x reference)
"""Trainium2 Bass kernel for the masked-logsumexp multi-branch loss.

Problem: logit [524288, 128] f32, target [524288, 128] int32 (multi-hot 0/1).
Per row r (is_null = target[r,0]):
  branch1 (is_null): lse(all) - x0
  branch2: (n_pos*lse({0} u pos) - sum_pos_logit)/max(n_pos,1) + lse(neg u {0}) - x0
Output: scalar mean over all rows.

Data-parallel over 8 NeuronCores (65536 rows each), rows on SBUF partitions.
Logits are N(0,1) so exp() cannot overflow; the three masked logsumexps share
unshifted per-row sums: S_E = sum exp(x), S_ME = sum M*exp(x),
S_MX = sum M*x, S_M = sum M, plus the class-0 column extracts.

Per [128, 32*128] super-tile (W5 = [mbf|pme|pmx|et|xbf] stacked in one tile):
  ScalarE : mask int32->bf16, exp(x)->bf16, x->bf16, column extracts
  VectorE : ONE fused dual product (broadcast mask AP x [et|xbf], bf16 2x);
            one shared IN-PLACE fold chain (seven 2x halving adds over
            [mask|M*E|M*x|E] written back into the same tile, last level
            lands in the bf16 stats tile)
  (GPSIMD/TensorE unused: Pool's shared SBUF port steals ~0.6ns/ns of DVE
   throughput - measured; PE can only contract the partition axis.)
Stats/combine run in bf16 (DVE 2x; final mean tolerance is 2e-2 and the
measured cost is ~3e-4); per-row losses are materialized fp32 and streamed
to HBM per combine chunk; the final mean is taken on host.
Ramp-up: the first three super-tiles run at 1/4 / 1/2 granularity with
split products so DVE starts ~14us earlier; the DMA stream is saturated
until ~st 4 (finer splitting past st 2 costs more instruction overhead
than the transition stall it removes - measured).

Measured on trn2 (8 cores): 257.4-262.4 us HW exec across runs, best
257.4 (vs 269.2 us for the NB=16 separate-buffer version), output rel
err ~3.5e-4. Engine busy: DVE ~237us span with <3us internal gaps (at
the 2x TT throughput bound for products+folds - TT caps at 2x because
both SBUF ports feed operands; only 1-input ops get 4x), ScalarE ~201us,
DMA ~190us active; ~12.5us head preamble (sem-range clears + engine
program loads) and ~5us tail barriers are framework-fixed.

Note: this container's walrus accepts one sync-wait per instruction, so
_split_sync_waits() rewrites the Tile-scheduled BIR accordingly.
"""
import numpy as np

import concourse.bass as bass
import concourse.tile as tile
from concourse import mybir
from concourse.bass_utils import run_bass_kernel_spmd

B = 524288
C = 128
NCORES = 8
RPC = B // NCORES  # rows per core = 65536
P = 128  # partitions
NB = 32  # class-blocks per super-tile -> [128, NB*128] tiles
ST = RPC // (P * NB)  # super-tiles per core = 16
NSTATS = ST * NB  # stat columns per core = 512

FP32 = mybir.dt.float32
BF16 = mybir.dt.bfloat16
I32 = mybir.dt.int32
ALU = mybir.AluOpType
AF = mybir.ActivationFunctionType


def _build_kernel(tc: tile.TileContext, lo, logit, target):
    nc = tc.nc
    # row = (s*128 + p)*NB + n ; per (s, p): NB*C contiguous f32 = 16KB DMA lines
    Xd = logit.rearrange("(s p n) c -> s p (n c)", p=P, n=NB)
    Md = target.rearrange("(s p n) c -> s p (n c)", p=P, n=NB)
    LOd = lo.rearrange("(s p n) -> p s n", p=P, n=NB)

    import contextlib

    with contextlib.ExitStack() as ctx:
        stats = ctx.enter_context(tc.tile_pool(name="stats", bufs=1))
        work = ctx.enter_context(tc.tile_pool(name="work", bufs=3))
        dmap = ctx.enter_context(tc.tile_pool(name="dmap", bufs=2))
        small = ctx.enter_context(tc.tile_pool(name="small", bufs=1))

        # Persistent per-core stat arrays: S_ALL[q] for q in [M, ME, MX, E]
        # bf16 stats keep the whole combine phase in DVE 2x mode; the
        # final per-row loss is materialized in fp32.
        S_ALL = stats.tile([P, 4, ST, NB], BF16)
        X0 = stats.tile([P, ST, NB], BF16)
        IS0 = stats.tile([P, ST, NB], BF16)

        lot = small.tile([P, NSTATS], FP32, tag="lot")

        # ---- combine (emitted interleaved below) ----
        # chunk boundaries in super-tiles; the last chunk is small so less
        # of it hangs past the end of the main loop
        CH_ENDS = (10, 16)

        def flat(t):
            return t.rearrange("p a b -> p (a b)")

        def combine_chunk(s_lo, s_hi):
            sl = slice(s_lo * NB, s_hi * NB)
            W = (s_hi - s_lo) * NB
            sM = flat(S_ALL[:, 0])[:, sl]
            sME = flat(S_ALL[:, 1])[:, sl]
            sMX = flat(S_ALL[:, 2])[:, sl]
            sE = flat(S_ALL[:, 3])[:, sl]
            x0 = flat(X0)[:, sl]
            m0 = flat(IS0)[:, sl]

            WMAX = 320

            def ctile(tag):
                full = small.tile([P, WMAX], BF16, tag=tag, name=f"ct_{tag}")
                return full[:, 0:W]

            E0 = ctile("c0")
            t_a = ctile("c1")
            t_b = ctile("c2")
            lse_all = ctile("c3")
            lse_pos = ctile("c4")
            lse_neg = ctile("c5")
            npos = ctile("c6")
            rinv = ctile("c7")
            acc = ctile("c8")

            # E0 = exp(x0)
            nc.scalar.activation(out=E0, in_=x0, func=AF.Exp)
            # t_a = S_ME - M0*E0   (= sum_{c>=1} M*E)
            nc.vector.tensor_mul(t_b, m0, E0)
            nc.vector.tensor_sub(t_a, sME, t_b)
            # lse_pos = log(E0 + t_a)
            nc.vector.tensor_add(t_b, t_a, E0)
            nc.scalar.activation(out=lse_pos, in_=t_b, func=AF.Ln)
            # lse_neg = log(max(S_E - t_a, E0))
            nc.vector.tensor_sub(t_b, sE, t_a)
            nc.vector.tensor_tensor(out=t_b, in0=t_b, in1=E0, op=ALU.max)
            nc.scalar.activation(out=lse_neg, in_=t_b, func=AF.Ln)
            # lse_all = log(S_E)
            nc.scalar.activation(out=lse_all, in_=sE, func=AF.Ln)
            # n_pos = S_M - M0 ; rinv = 1/max(n_pos, 1)
            nc.vector.tensor_sub(npos, sM, m0)
            nc.vector.tensor_scalar_max(t_b, npos, 1.0)
            # 1/n via ScalarE: exp(-ln n) (same ACT table set as Exp/Ln)
            nc.scalar.activation(out=rinv, in_=t_b, func=AF.Ln)
            nc.scalar.activation(out=rinv, in_=rinv, func=AF.Exp, scale=-1.0)
            # acc = (n_pos*lse_pos - (S_MX - M0*x0)) * rinv + lse_neg
            nc.vector.tensor_mul(t_b, m0, x0)
            nc.vector.tensor_sub(t_b, sMX, t_b)
            nc.vector.tensor_mul(t_a, npos, lse_pos)
            nc.vector.tensor_sub(t_a, t_a, t_b)
            nc.vector.tensor_mul(t_a, t_a, rinv)
            nc.vector.tensor_add(acc, t_a, lse_neg)
            # lo = M0*(lse_all - acc) + acc - x0
            nc.vector.tensor_sub(t_a, lse_all, acc)
            nc.vector.tensor_mul(t_a, t_a, m0)
            nc.vector.tensor_add(t_a, t_a, acc)
            nc.vector.tensor_sub(lot[:, sl], t_a, x0)

        def process_slice(s, xt, mt, W5, n0, n1, split_prod=False):
            """Emit the compute pipeline for class-blocks [n0, n1) of
            super-tile s (mask DMA'd first so ScalarE starts earliest)."""
            nn = slice(n0, n1)
            mbf = W5[:, 0, nn]
            et = W5[:, 3, nn]
            xbf = W5[:, 4, nn]
            Q = W5[:, 0:4, nn]

            # ScalarE: int32 -> bf16 mask convert ; exp ; bf16 logits
            nc.scalar.copy(out=mbf, in_=mt[:, nn])
            nc.scalar.activation(out=et, in_=xt[:, nn], func=AF.Exp)
            if split_prod:
                # ramp: DVE starts on M*E right after the exp; M*x reads
                # the f32 logits directly (1x, but DVE idles here anyway)
                # so ScalarE's serial feed chain is 2 passes instead of 3
                nc.vector.tensor_mul(W5[:, 1, nn], et, mbf)
                nc.vector.tensor_mul(W5[:, 2, nn], mbf, xt[:, nn])
            else:
                nc.scalar.copy(out=xbf, in_=xt[:, nn])
            # column extracts (class 0) on ScalarE
            nc.scalar.copy(out=X0[:, s, nn], in_=xt[:, nn, 0])
            nc.scalar.copy(out=IS0[:, s, nn], in_=mt[:, nn, 0])

            if not split_prod:
                # ONE fused dual product: [pme|pmx] = bcast(mbf) * [et|xbf]
                nc.vector.tensor_tensor(
                    out=W5[:, 1:3, nn],
                    in0=mbf[:, None].broadcast_to([P, 2, n1 - n0, C]),
                    in1=W5[:, 3:5, nn],
                    op=ALU.mult,
                )
            # In-place fold chain over the four streams (bf16 2x halving
            # adds); the last level lands directly in the bf16 stats tile
            for w in (64, 32, 16, 8, 4, 2):
                nc.vector.tensor_add(
                    Q[:, :, :, 0:w], Q[:, :, :, 0:w], Q[:, :, :, w : 2 * w]
                )
            nc.vector.tensor_add(
                S_ALL[:, :, s, nn], Q[:, :, :, 0], Q[:, :, :, 1]
            )

        for s in range(ST):
            xt = dmap.tile([P, NB, C], FP32, tag="xt")
            mt = dmap.tile([P, NB, C], I32, tag="mt")
            Xs = Xd[s].rearrange("p (n c) -> p n c", c=C)
            Ms = Md[s].rearrange("p (n c) -> p n c", c=C)
            W5 = work.tile([P, 5, NB, C], BF16, tag="W5")
            if s < 3:
                # ramp-up: finer-granular DMA + compute while the DMA
                # stream is still behind the consumption front; the very
                # first two slices are eighths so compute starts earliest
                bounds = (
                    (0, 4, 8, 16, 24, 32) if s == 0 else (0, 16, 32)
                )
                for n0, n1 in zip(bounds, bounds[1:]):
                    nn = slice(n0, n1)
                    nc.sync.dma_start(out=mt[:, nn], in_=Ms[:, nn])
                    nc.sync.dma_start(out=xt[:, nn], in_=Xs[:, nn])
                    process_slice(s, xt, mt, W5, n0, n1, split_prod=True)
            else:
                nc.sync.dma_start(out=mt, in_=Ms)
                nc.sync.dma_start(out=xt, in_=Xs)
                process_slice(s, xt, mt, W5, 0, NB)
            if s + 1 in CH_ENDS:
                s_lo = 0 if s + 1 == CH_ENDS[0] else CH_ENDS[CH_ENDS.index(s + 1) - 1]
                combine_chunk(s_lo, s + 1)
                # stream the per-chunk losses out as soon as they're ready
                nc.sync.dma_start(
                    out=LOd[:, s_lo : s + 1, :],
                    in_=lot[:, s_lo * NB : (s + 1) * NB].rearrange(
                        "p (s n) -> p s n", n=NB
                    ),
                )


def _split_sync_waits(nc):
    """The container's walrus accepts at most ONE sync-wait command per
    instruction (the TPB EVENTS struct has a single wait slot). Tile emits
    instructions with N waits; rewrite each so the extra waits ride on
    same-engine NoOps inserted immediately before (engine program order makes
    this semantically identical)."""
    for f in nc.m.functions:
        for blk in f.blocks:
            insts = blk.instructions
            out = []
            changed = False
            for inst in insts:
                si = inst.sync_info
                waits = list(si.on_wait) if (si is not None and si.on_wait) else []
                if len(waits) > 1:
                    changed = True
                    for k, w in enumerate(waits[:-1]):
                        nop = mybir.InstNoOp(name=f"{inst.name}-w{k}", ins=[], outs=[])
                        nop.engine = inst.engine
                        nop.sync_info = mybir.SyncInfo(on_wait=[w], on_update=[])
                        out.append(nop)
                    inst.sync_info = mybir.SyncInfo(
                        on_wait=[waits[-1]],
                        on_update=list(si.on_update) if si.on_update else [],
                    )
                out.append(inst)
            if changed:
                blk.instructions = out


_NC_CACHE = None
SPLIT_WAITS = True


def _get_nc():
    global _NC_CACHE
    if _NC_CACHE is None:
        nc = bass.Bass()
        logit = nc.declare_dram_parameter("logit", [RPC, C], FP32, isOutput=False)
        target = nc.declare_dram_parameter("target", [RPC, C], I32, isOutput=False)
        lo = nc.declare_dram_parameter("lo", [RPC], FP32, isOutput=True)
        with tile.TileContext(nc) as tc:
            _build_kernel(tc, lo, logit, target)
        if SPLIT_WAITS:
            _split_sync_waits(nc)
        _NC_CACHE = nc
    return _NC_CACHE


def kernel(**inputs) -> np.ndarray:
    logit = np.ascontiguousarray(np.asarray(inputs["logit"], dtype=np.float32))
    target = np.ascontiguousarray(np.asarray(inputs["target"], dtype=np.int32))
    assert logit.shape == (B, C) and target.shape == (B, C)

    nc = _get_nc()
    in_maps = [
        {
            "logit": logit[i * RPC : (i + 1) * RPC],
            "target": target[i * RPC : (i + 1) * RPC],
        }
        for i in range(NCORES)
    ]
    res = run_bass_kernel_spmd(nc, in_maps, core_ids=list(range(NCORES)))
    lo = np.concatenate([r["lo"].reshape(-1) for r in res.results])
    return np.array(np.mean(lo, dtype=np.float64), dtype=np.float32)


# revision 36
# speedup vs baseline: 1.0048x; 1.0048x over previous
"""Trainium2 Bass kernel for the masked-logsumexp multi-branch loss.

Problem: logit [524288, 128] f32, target [524288, 128] int32 (multi-hot 0/1).
Per row r (is_null = target[r,0]):
  branch1 (is_null): lse(all) - x0
  branch2: (n_pos*lse({0} u pos) - sum_pos_logit)/max(n_pos,1) + lse(neg u {0}) - x0
Output: scalar mean over all rows.

Data-parallel over 8 NeuronCores (65536 rows each), rows on SBUF partitions.
Logits are N(0,1) so exp() cannot overflow; the three masked logsumexps share
unshifted per-row sums: S_E = sum exp(x), S_ME = sum M*exp(x),
S_MX = sum M*x, S_M = sum M, plus the class-0 column extracts.

Per [128, 32*128] super-tile (W5 = [mbf|pme|pmx|et|xbf] stacked in one tile):
  ScalarE : mask int32->bf16, exp(x)->bf16, x->bf16, column extracts
  VectorE : ONE fused dual product (broadcast mask AP x [et|xbf], bf16 2x);
            one shared IN-PLACE fold chain (seven 2x halving adds over
            [mask|M*E|M*x|E] written back into the same tile, last level
            lands in the bf16 stats tile)
  (GPSIMD/TensorE unused: Pool's shared SBUF port steals ~0.6ns/ns of DVE
   throughput - measured; PE can only contract the partition axis.)
Stats/combine run in bf16 (DVE 2x; final mean tolerance is 2e-2 and the
measured cost is ~3e-4); per-row losses are materialized fp32 and streamed
to HBM per combine chunk; the final mean is taken on host.
Ramp-up: the first three super-tiles run at 1/4 / 1/2 granularity with
split products so DVE starts ~14us earlier; the DMA stream is saturated
until ~st 4 (finer splitting past st 2 costs more instruction overhead
than the transition stall it removes - measured).

Measured on trn2 (8 cores): 257.4-262.4 us HW exec across runs, best
257.4 (vs 269.2 us for the NB=16 separate-buffer version), output rel
err ~3.5e-4. Engine busy: DVE ~237us span with <3us internal gaps (at
the 2x TT throughput bound for products+folds - TT caps at 2x because
both SBUF ports feed operands; only 1-input ops get 4x), ScalarE ~201us,
DMA ~190us active; ~12.5us head preamble (sem-range clears + engine
program loads) and ~5us tail barriers are framework-fixed.

Note: this container's walrus accepts one sync-wait per instruction, so
_split_sync_waits() rewrites the Tile-scheduled BIR accordingly.
"""
import numpy as np

import concourse.bass as bass
import concourse.tile as tile
from concourse import mybir
from concourse.bass_utils import run_bass_kernel_spmd

B = 524288
C = 128
NCORES = 8
RPC = B // NCORES  # rows per core = 65536
P = 128  # partitions
NB = 32  # class-blocks per super-tile -> [128, NB*128] tiles
ST = RPC // (P * NB)  # super-tiles per core = 16
NSTATS = ST * NB  # stat columns per core = 512

FP32 = mybir.dt.float32
BF16 = mybir.dt.bfloat16
I32 = mybir.dt.int32
ALU = mybir.AluOpType
AF = mybir.ActivationFunctionType


def _build_kernel(tc: tile.TileContext, lo, logit, target):
    nc = tc.nc
    # row = (s*128 + p)*NB + n ; per (s, p): NB*C contiguous f32 = 16KB DMA lines
    Xd = logit.rearrange("(s p n) c -> s p (n c)", p=P, n=NB)
    Md = target.rearrange("(s p n) c -> s p (n c)", p=P, n=NB)
    LOd = lo.rearrange("(s p n) -> p s n", p=P, n=NB)

    import contextlib

    with contextlib.ExitStack() as ctx:
        stats = ctx.enter_context(tc.tile_pool(name="stats", bufs=1))
        work = ctx.enter_context(tc.tile_pool(name="work", bufs=3))
        dmap = ctx.enter_context(tc.tile_pool(name="dmap", bufs=2))
        small = ctx.enter_context(tc.tile_pool(name="small", bufs=1))

        # Persistent per-core stat arrays: S_ALL[q] for q in [M, ME, MX, E]
        # bf16 stats keep the whole combine phase in DVE 2x mode; the
        # final per-row loss is materialized in fp32.
        S_ALL = stats.tile([P, 4, ST, NB], BF16)
        X0 = stats.tile([P, ST, NB], BF16)
        IS0 = stats.tile([P, ST, NB], BF16)

        lot = small.tile([P, NSTATS], FP32, tag="lot")

        # ---- combine (emitted interleaved below) ----
        # chunk boundaries in super-tiles; the last chunk is small so less
        # of it hangs past the end of the main loop
        CH_ENDS = (10, 16)

        def flat(t):
            return t.rearrange("p a b -> p (a b)")

        def combine_chunk(s_lo, s_hi):
            sl = slice(s_lo * NB, s_hi * NB)
            W = (s_hi - s_lo) * NB
            sM = flat(S_ALL[:, 0])[:, sl]
            sME = flat(S_ALL[:, 1])[:, sl]
            sMX = flat(S_ALL[:, 2])[:, sl]
            sE = flat(S_ALL[:, 3])[:, sl]
            x0 = flat(X0)[:, sl]
            m0 = flat(IS0)[:, sl]

            WMAX = 320

            def ctile(tag):
                full = small.tile([P, WMAX], BF16, tag=tag, name=f"ct_{tag}")
                return full[:, 0:W]

            E0 = ctile("c0")
            t_a = ctile("c1")
            t_b = ctile("c2")
            lse_all = ctile("c3")
            lse_pos = ctile("c4")
            lse_neg = ctile("c5")
            npos = ctile("c6")
            rinv = ctile("c7")
            acc = ctile("c8")

            # E0 = exp(x0)
            nc.scalar.activation(out=E0, in_=x0, func=AF.Exp)
            # t_a = S_ME - M0*E0   (= sum_{c>=1} M*E)
            nc.vector.tensor_mul(t_b, m0, E0)
            nc.vector.tensor_sub(t_a, sME, t_b)
            # lse_pos = log(E0 + t_a)
            nc.vector.tensor_add(t_b, t_a, E0)
            nc.scalar.activation(out=lse_pos, in_=t_b, func=AF.Ln)
            # lse_neg = log(max(S_E - t_a, E0))
            nc.vector.tensor_sub(t_b, sE, t_a)
            nc.vector.tensor_tensor(out=t_b, in0=t_b, in1=E0, op=ALU.max)
            nc.scalar.activation(out=lse_neg, in_=t_b, func=AF.Ln)
            # lse_all = log(S_E)
            nc.scalar.activation(out=lse_all, in_=sE, func=AF.Ln)
            # n_pos = S_M - M0 ; rinv = 1/max(n_pos, 1)
            nc.vector.tensor_sub(npos, sM, m0)
            nc.vector.tensor_scalar_max(t_b, npos, 1.0)
            # 1/n via ScalarE: exp(-ln n) (same ACT table set as Exp/Ln)
            nc.scalar.activation(out=rinv, in_=t_b, func=AF.Ln)
            nc.scalar.activation(out=rinv, in_=rinv, func=AF.Exp, scale=-1.0)
            # acc = (n_pos*lse_pos - (S_MX - M0*x0)) * rinv + lse_neg
            nc.vector.tensor_mul(t_b, m0, x0)
            nc.vector.tensor_sub(t_b, sMX, t_b)
            nc.vector.tensor_mul(t_a, npos, lse_pos)
            nc.vector.tensor_sub(t_a, t_a, t_b)
            nc.vector.tensor_mul(t_a, t_a, rinv)
            nc.vector.tensor_add(acc, t_a, lse_neg)
            # lo = M0*(lse_all - acc) + acc - x0
            nc.vector.tensor_sub(t_a, lse_all, acc)
            nc.vector.tensor_mul(t_a, t_a, m0)
            nc.vector.tensor_add(t_a, t_a, acc)
            nc.vector.tensor_sub(lot[:, sl], t_a, x0)

        def process_slice(s, xt, mt, W5, n0, n1, split_prod=False, ramp_1x=False):
            """Emit the compute pipeline for class-blocks [n0, n1) of
            super-tile s (mask DMA'd first so ScalarE starts earliest)."""
            nn = slice(n0, n1)
            mbf = W5[:, 0, nn]
            et = W5[:, 3, nn]
            xbf = W5[:, 4, nn]
            Q = W5[:, 0:4, nn]

            # ScalarE: int32 -> bf16 mask convert ; exp ; bf16 logits
            nc.scalar.copy(out=mbf, in_=mt[:, nn])
            nc.scalar.activation(out=et, in_=xt[:, nn], func=AF.Exp)
            if ramp_1x:
                # first slices: DVE starts on M*E right after the exp; M*x
                # reads the f32 logits directly (1x, but DVE idles here
                # anyway) so ScalarE's serial feed is 2 passes instead of 3
                nc.vector.tensor_mul(W5[:, 1, nn], et, mbf)
                nc.vector.tensor_mul(W5[:, 2, nn], mbf, xt[:, nn])
            elif split_prod:
                nc.vector.tensor_mul(W5[:, 1, nn], et, mbf)
                nc.scalar.copy(out=xbf, in_=xt[:, nn])
                nc.vector.tensor_mul(W5[:, 2, nn], xbf, mbf)
            else:
                nc.scalar.copy(out=xbf, in_=xt[:, nn])
            # column extracts (class 0) on ScalarE
            nc.scalar.copy(out=X0[:, s, nn], in_=xt[:, nn, 0])
            nc.scalar.copy(out=IS0[:, s, nn], in_=mt[:, nn, 0])

            if not (split_prod or ramp_1x):
                # ONE fused dual product: [pme|pmx] = bcast(mbf) * [et|xbf]
                nc.vector.tensor_tensor(
                    out=W5[:, 1:3, nn],
                    in0=mbf[:, None].broadcast_to([P, 2, n1 - n0, C]),
                    in1=W5[:, 3:5, nn],
                    op=ALU.mult,
                )
            # In-place fold chain over the four streams (bf16 2x halving
            # adds); the last level lands directly in the bf16 stats tile
            for w in (64, 32, 16, 8, 4, 2):
                nc.vector.tensor_add(
                    Q[:, :, :, 0:w], Q[:, :, :, 0:w], Q[:, :, :, w : 2 * w]
                )
            nc.vector.tensor_add(
                S_ALL[:, :, s, nn], Q[:, :, :, 0], Q[:, :, :, 1]
            )

        for s in range(ST):
            xt = dmap.tile([P, NB, C], FP32, tag="xt")
            mt = dmap.tile([P, NB, C], I32, tag="mt")
            Xs = Xd[s].rearrange("p (n c) -> p n c", c=C)
            Ms = Md[s].rearrange("p (n c) -> p n c", c=C)
            W5 = work.tile([P, 5, NB, C], BF16, tag="W5")
            if s < 3:
                # ramp-up: finer-granular DMA + compute while the DMA
                # stream is still behind the consumption front; the very
                # first two slices are eighths so compute starts earliest
                bounds = (
                    (0, 4, 8, 16, 24, 32) if s == 0 else (0, 16, 32)
                )
                for qi, (n0, n1) in enumerate(zip(bounds, bounds[1:])):
                    nn = slice(n0, n1)
                    nc.sync.dma_start(out=mt[:, nn], in_=Ms[:, nn])
                    nc.sync.dma_start(out=xt[:, nn], in_=Xs[:, nn])
                    process_slice(
                        s, xt, mt, W5, n0, n1,
                        split_prod=True,
                        ramp_1x=(s == 0 and qi < 3),
                    )
            else:
                nc.sync.dma_start(out=mt, in_=Ms)
                nc.sync.dma_start(out=xt, in_=Xs)
                process_slice(s, xt, mt, W5, 0, NB)
            if s + 1 in CH_ENDS:
                s_lo = 0 if s + 1 == CH_ENDS[0] else CH_ENDS[CH_ENDS.index(s + 1) - 1]
                combine_chunk(s_lo, s + 1)
                # stream the per-chunk losses out as soon as they're ready
                nc.sync.dma_start(
                    out=LOd[:, s_lo : s + 1, :],
                    in_=lot[:, s_lo * NB : (s + 1) * NB].rearrange(
                        "p (s n) -> p s n", n=NB
                    ),
                )


def _split_sync_waits(nc):
    """The container's walrus accepts at most ONE sync-wait command per
    instruction (the TPB EVENTS struct has a single wait slot). Tile emits
    instructions with N waits; rewrite each so the extra waits ride on
    same-engine NoOps inserted immediately before (engine program order makes
    this semantically identical)."""
    for f in nc.m.functions:
        for blk in f.blocks:
            insts = blk.instructions
            out = []
            changed = False
            for inst in insts:
                si = inst.sync_info
                waits = list(si.on_wait) if (si is not None and si.on_wait) else []
                if len(waits) > 1:
                    changed = True
                    for k, w in enumerate(waits[:-1]):
                        nop = mybir.InstNoOp(name=f"{inst.name}-w{k}", ins=[], outs=[])
                        nop.engine = inst.engine
                        nop.sync_info = mybir.SyncInfo(on_wait=[w], on_update=[])
                        out.append(nop)
                    inst.sync_info = mybir.SyncInfo(
                        on_wait=[waits[-1]],
                        on_update=list(si.on_update) if si.on_update else [],
                    )
                out.append(inst)
            if changed:
                blk.instructions = out


_NC_CACHE = None
SPLIT_WAITS = True


def _get_nc():
    global _NC_CACHE
    if _NC_CACHE is None:
        nc = bass.Bass()
        logit = nc.declare_dram_parameter("logit", [RPC, C], FP32, isOutput=False)
        target = nc.declare_dram_parameter("target", [RPC, C], I32, isOutput=False)
        lo = nc.declare_dram_parameter("lo", [RPC], FP32, isOutput=True)
        with tile.TileContext(nc) as tc:
            _build_kernel(tc, lo, logit, target)
        if SPLIT_WAITS:
            _split_sync_waits(nc)
        _NC_CACHE = nc
    return _NC_CACHE


def kernel(**inputs) -> np.ndarray:
    logit = np.ascontiguousarray(np.asarray(inputs["logit"], dtype=np.float32))
    target = np.ascontiguousarray(np.asarray(inputs["target"], dtype=np.int32))
    assert logit.shape == (B, C) and target.shape == (B, C)

    nc = _get_nc()
    in_maps = [
        {
            "logit": logit[i * RPC : (i + 1) * RPC],
            "target": target[i * RPC : (i + 1) * RPC],
        }
        for i in range(NCORES)
    ]
    res = run_bass_kernel_spmd(nc, in_maps, core_ids=list(range(NCORES)))
    lo = np.concatenate([r["lo"].reshape(-1) for r in res.results])
    return np.array(np.mean(lo, dtype=np.float64), dtype=np.float32)


# revision 37
# speedup vs baseline: 1.0205x; 1.0157x over previous
"""Trainium2 Bass kernel for the masked-logsumexp multi-branch loss.

Problem: logit [524288, 128] f32, target [524288, 128] int32 (multi-hot 0/1).
Per row r (is_null = target[r,0]):
  branch1 (is_null): lse(all) - x0
  branch2: (n_pos*lse({0} u pos) - sum_pos_logit)/max(n_pos,1) + lse(neg u {0}) - x0
Output: scalar mean over all rows.

Data-parallel over 8 NeuronCores (65536 rows each), rows on SBUF partitions.
Logits are N(0,1) so exp() cannot overflow; the three masked logsumexps share
unshifted per-row sums: S_E = sum exp(x), S_ME = sum M*exp(x),
S_MX = sum M*x, S_M = sum M, plus the class-0 column extracts.

Per [128, 32*128] super-tile (W5 = [mbf|pme|pmx|et|xbf] stacked in one tile):
  ScalarE : mask int32->bf16, exp(x)->bf16, x->bf16, column extracts
  VectorE : ONE fused dual product (broadcast mask AP x [et|xbf], bf16 2x);
            one shared IN-PLACE fold chain (seven 2x halving adds over
            [mask|M*E|M*x|E] written back into the same tile, last level
            lands in the bf16 stats tile)
  (GPSIMD/TensorE unused: Pool's shared SBUF port steals ~0.6ns/ns of DVE
   throughput - measured; PE can only contract the partition axis.)
Stats/combine run in bf16 (DVE 2x; final mean tolerance is 2e-2 and the
measured cost is ~3e-4); per-row losses are materialized fp32 and streamed
to HBM per combine chunk; the final mean is taken on host.
Ramp-up: the first three super-tiles run at 1/4 / 1/2 granularity with
split products so DVE starts ~14us earlier; the DMA stream is saturated
until ~st 4 (finer splitting past st 2 costs more instruction overhead
than the transition stall it removes - measured).

Measured on trn2 (8 cores): 257.4-262.4 us HW exec across runs, best
257.4 (vs 269.2 us for the NB=16 separate-buffer version), output rel
err ~3.5e-4. Engine busy: DVE ~237us span with <3us internal gaps (at
the 2x TT throughput bound for products+folds - TT caps at 2x because
both SBUF ports feed operands; only 1-input ops get 4x), ScalarE ~201us,
DMA ~190us active; ~12.5us head preamble (sem-range clears + engine
program loads) and ~5us tail barriers are framework-fixed.

Note: this container's walrus accepts one sync-wait per instruction, so
_split_sync_waits() rewrites the Tile-scheduled BIR accordingly.
"""
import numpy as np

import concourse.bass as bass
import concourse.tile as tile
from concourse import mybir
from concourse.bass_utils import run_bass_kernel_spmd

B = 524288
C = 128
NCORES = 8
RPC = B // NCORES  # rows per core = 65536
P = 128  # partitions
NB = 32  # class-blocks per super-tile -> [128, NB*128] tiles
ST = RPC // (P * NB)  # super-tiles per core = 16
NSTATS = ST * NB  # stat columns per core = 512

FP32 = mybir.dt.float32
BF16 = mybir.dt.bfloat16
I32 = mybir.dt.int32
ALU = mybir.AluOpType
AF = mybir.ActivationFunctionType


def _build_kernel(tc: tile.TileContext, lo, logit, target):
    nc = tc.nc
    # row = (s*128 + p)*NB + n ; per (s, p): NB*C contiguous f32 = 16KB DMA lines
    Xd = logit.rearrange("(s p n) c -> s p (n c)", p=P, n=NB)
    Md = target.rearrange("(s p n) c -> s p (n c)", p=P, n=NB)
    LOd = lo.rearrange("(s p n) -> p s n", p=P, n=NB)

    import contextlib

    with contextlib.ExitStack() as ctx:
        stats = ctx.enter_context(tc.tile_pool(name="stats", bufs=1))
        work = ctx.enter_context(tc.tile_pool(name="work", bufs=3))
        dmap = ctx.enter_context(tc.tile_pool(name="dmap", bufs=2))
        small = ctx.enter_context(tc.tile_pool(name="small", bufs=1))

        # Persistent per-core stat arrays: S_ALL[q] for q in [M, ME, MX, E]
        # bf16 stats keep the whole combine phase in DVE 2x mode; the
        # final per-row loss is materialized in fp32.
        S_ALL = stats.tile([P, 4, ST, NB], BF16)
        X0 = stats.tile([P, ST, NB], BF16)
        IS0 = stats.tile([P, ST, NB], BF16)

        lot = small.tile([P, NSTATS], FP32, tag="lot")

        # ---- combine (emitted interleaved below) ----
        # chunk boundaries in super-tiles; the last chunk is small so less
        # of it hangs past the end of the main loop
        CH_ENDS = (10, 16)

        def flat(t):
            return t.rearrange("p a b -> p (a b)")

        def combine_chunk(s_lo, s_hi):
            sl = slice(s_lo * NB, s_hi * NB)
            W = (s_hi - s_lo) * NB
            sM = flat(S_ALL[:, 0])[:, sl]
            sME = flat(S_ALL[:, 1])[:, sl]
            sMX = flat(S_ALL[:, 2])[:, sl]
            sE = flat(S_ALL[:, 3])[:, sl]
            x0 = flat(X0)[:, sl]
            m0 = flat(IS0)[:, sl]

            WMAX = 320

            def ctile(tag):
                full = small.tile([P, WMAX], BF16, tag=tag, name=f"ct_{tag}")
                return full[:, 0:W]

            E0 = ctile("c0")
            t_a = ctile("c1")
            t_b = ctile("c2")
            lse_all = ctile("c3")
            lse_pos = ctile("c4")
            lse_neg = ctile("c5")
            npos = ctile("c6")
            rinv = ctile("c7")
            acc = ctile("c8")

            # E0 = exp(x0)
            nc.scalar.activation(out=E0, in_=x0, func=AF.Exp)
            # t_a = S_ME - M0*E0   (= sum_{c>=1} M*E)
            nc.vector.tensor_mul(t_b, m0, E0)
            nc.vector.tensor_sub(t_a, sME, t_b)
            # lse_pos = log(E0 + t_a)
            nc.vector.tensor_add(t_b, t_a, E0)
            nc.scalar.activation(out=lse_pos, in_=t_b, func=AF.Ln)
            # lse_neg = log(max(S_E - t_a, E0))
            nc.vector.tensor_sub(t_b, sE, t_a)
            nc.vector.tensor_tensor(out=t_b, in0=t_b, in1=E0, op=ALU.max)
            nc.scalar.activation(out=lse_neg, in_=t_b, func=AF.Ln)
            # lse_all = log(S_E)
            nc.scalar.activation(out=lse_all, in_=sE, func=AF.Ln)
            # n_pos = S_M - M0 ; rinv = 1/max(n_pos, 1)
            nc.vector.tensor_sub(npos, sM, m0)
            nc.vector.tensor_scalar_max(t_b, npos, 1.0)
            # 1/n via ScalarE: exp(-ln n) (same ACT table set as Exp/Ln)
            nc.scalar.activation(out=rinv, in_=t_b, func=AF.Ln)
            nc.scalar.activation(out=rinv, in_=rinv, func=AF.Exp, scale=-1.0)
            # acc = (n_pos*lse_pos - (S_MX - M0*x0)) * rinv + lse_neg
            nc.vector.tensor_mul(t_b, m0, x0)
            nc.vector.tensor_sub(t_b, sMX, t_b)
            nc.vector.tensor_mul(t_a, npos, lse_pos)
            nc.vector.tensor_sub(t_a, t_a, t_b)
            nc.vector.tensor_mul(t_a, t_a, rinv)
            nc.vector.tensor_add(acc, t_a, lse_neg)
            # lo = M0*(lse_all - acc) + acc - x0
            nc.vector.tensor_sub(t_a, lse_all, acc)
            nc.vector.tensor_mul(t_a, t_a, m0)
            nc.vector.tensor_add(t_a, t_a, acc)
            nc.vector.tensor_sub(lot[:, sl], t_a, x0)

        def process_slice(s, xt, mt, W5, n0, n1, split_prod=False, ramp_1x=False):
            """Emit the compute pipeline for class-blocks [n0, n1) of
            super-tile s (mask DMA'd first so ScalarE starts earliest)."""
            nn = slice(n0, n1)
            mbf = W5[:, 0, nn]
            et = W5[:, 3, nn]
            xbf = W5[:, 4, nn]
            Q = W5[:, 0:4, nn]

            # ScalarE: int32 -> bf16 mask convert ; exp ; bf16 logits
            nc.scalar.copy(out=mbf, in_=mt[:, nn])
            nc.scalar.activation(out=et, in_=xt[:, nn], func=AF.Exp)
            if ramp_1x:
                # first slices: DVE starts on M*E right after the exp; M*x
                # reads the f32 logits directly (1x, but DVE idles here
                # anyway) so ScalarE's serial feed is 2 passes instead of 3
                nc.vector.tensor_mul(W5[:, 1, nn], et, mbf)
                nc.vector.tensor_mul(W5[:, 2, nn], mbf, xt[:, nn])
            elif split_prod:
                nc.vector.tensor_mul(W5[:, 1, nn], et, mbf)
                nc.scalar.copy(out=xbf, in_=xt[:, nn])
                nc.vector.tensor_mul(W5[:, 2, nn], xbf, mbf)
            else:
                nc.scalar.copy(out=xbf, in_=xt[:, nn])
            # column extracts (class 0) on ScalarE
            nc.scalar.copy(out=X0[:, s, nn], in_=xt[:, nn, 0])
            nc.scalar.copy(out=IS0[:, s, nn], in_=mt[:, nn, 0])

            if not (split_prod or ramp_1x):
                # ONE fused dual product: [pme|pmx] = bcast(mbf) * [et|xbf]
                nc.vector.tensor_tensor(
                    out=W5[:, 1:3, nn],
                    in0=mbf[:, None].broadcast_to([P, 2, n1 - n0, C]),
                    in1=W5[:, 3:5, nn],
                    op=ALU.mult,
                )
            # In-place fold chain over the four streams (bf16 2x halving
            # adds); the last level lands directly in the bf16 stats tile
            for w in (64, 32, 16, 8, 4, 2):
                nc.vector.tensor_add(
                    Q[:, :, :, 0:w], Q[:, :, :, 0:w], Q[:, :, :, w : 2 * w]
                )
            nc.vector.tensor_add(
                S_ALL[:, :, s, nn], Q[:, :, :, 0], Q[:, :, :, 1]
            )

        for s in range(ST):
            xt = dmap.tile([P, NB, C], FP32, tag="xt")
            mt = dmap.tile([P, NB, C], I32, tag="mt")
            Xs = Xd[s].rearrange("p (n c) -> p n c", c=C)
            Ms = Md[s].rearrange("p (n c) -> p n c", c=C)
            W5 = work.tile([P, 5, NB, C], BF16, tag="W5")
            if s < 3:
                # ramp-up: finer-granular DMA + compute while the DMA
                # stream is still behind the consumption front; the very
                # first two slices are eighths so compute starts earliest
                bounds = (
                    (0, 8, 16, 24, 32) if s == 0 else (0, 16, 32)
                )
                for n0, n1 in zip(bounds, bounds[1:]):
                    nn = slice(n0, n1)
                    nc.sync.dma_start(out=mt[:, nn], in_=Ms[:, nn])
                    nc.sync.dma_start(out=xt[:, nn], in_=Xs[:, nn])
                    process_slice(s, xt, mt, W5, n0, n1)
            else:
                nc.sync.dma_start(out=mt, in_=Ms)
                nc.sync.dma_start(out=xt, in_=Xs)
                process_slice(s, xt, mt, W5, 0, NB)
            if s + 1 in CH_ENDS:
                s_lo = 0 if s + 1 == CH_ENDS[0] else CH_ENDS[CH_ENDS.index(s + 1) - 1]
                combine_chunk(s_lo, s + 1)
                # stream the per-chunk losses out as soon as they're ready
                nc.sync.dma_start(
                    out=LOd[:, s_lo : s + 1, :],
                    in_=lot[:, s_lo * NB : (s + 1) * NB].rearrange(
                        "p (s n) -> p s n", n=NB
                    ),
                )


def _split_sync_waits(nc):
    """The container's walrus accepts at most ONE sync-wait command per
    instruction (the TPB EVENTS struct has a single wait slot). Tile emits
    instructions with N waits; rewrite each so the extra waits ride on
    same-engine NoOps inserted immediately before (engine program order makes
    this semantically identical)."""
    for f in nc.m.functions:
        for blk in f.blocks:
            insts = blk.instructions
            out = []
            changed = False
            for inst in insts:
                si = inst.sync_info
                waits = list(si.on_wait) if (si is not None and si.on_wait) else []
                if len(waits) > 1:
                    changed = True
                    for k, w in enumerate(waits[:-1]):
                        nop = mybir.InstNoOp(name=f"{inst.name}-w{k}", ins=[], outs=[])
                        nop.engine = inst.engine
                        nop.sync_info = mybir.SyncInfo(on_wait=[w], on_update=[])
                        out.append(nop)
                    inst.sync_info = mybir.SyncInfo(
                        on_wait=[waits[-1]],
                        on_update=list(si.on_update) if si.on_update else [],
                    )
                out.append(inst)
            if changed:
                blk.instructions = out


_NC_CACHE = None
SPLIT_WAITS = True


def _get_nc():
    global _NC_CACHE
    if _NC_CACHE is None:
        nc = bass.Bass()
        logit = nc.declare_dram_parameter("logit", [RPC, C], FP32, isOutput=False)
        target = nc.declare_dram_parameter("target", [RPC, C], I32, isOutput=False)
        lo = nc.declare_dram_parameter("lo", [RPC], FP32, isOutput=True)
        with tile.TileContext(nc) as tc:
            _build_kernel(tc, lo, logit, target)
        if SPLIT_WAITS:
            _split_sync_waits(nc)
        _NC_CACHE = nc
    return _NC_CACHE


def kernel(**inputs) -> np.ndarray:
    logit = np.ascontiguousarray(np.asarray(inputs["logit"], dtype=np.float32))
    target = np.ascontiguousarray(np.asarray(inputs["target"], dtype=np.int32))
    assert logit.shape == (B, C) and target.shape == (B, C)

    nc = _get_nc()
    in_maps = [
        {
            "logit": logit[i * RPC : (i + 1) * RPC],
            "target": target[i * RPC : (i + 1) * RPC],
        }
        for i in range(NCORES)
    ]
    res = run_bass_kernel_spmd(nc, in_maps, core_ids=list(range(NCORES)))
    lo = np.concatenate([r["lo"].reshape(-1) for r in res.results])
    return np.array(np.mean(lo, dtype=np.float64), dtype=np.float32)
